# revision 21
# baseline (speedup 1.0000x reference)
"""Trainium2 Bass kernel for DepthwiseXCorr (SiamRPN-style) model.

Pipeline (per sample): conv3x3+BN+ReLU on kernel & search branches,
depthwise cross-correlation, 1x1 conv + BN + ReLU head, 1x1 conv + bias.

Sharding: data-parallel over batch across 8 NeuronCores (8 samples each),
weights replicated.  BN is folded into conv weights on the host.

Layout on device: channels on SBUF partitions (2 chunks of 128), spatial x
batch on the free dimension.  Convolutions run as 9 shifted matmuls in bf16
(split-row PSUM tiles interleaved so consecutive matmuls share weights).
The depthwise xcorr is elementwise work spread over three engines per
(sample, channel-chunk) pair: 'v' pairs run multiply-accumulate chains on
the VectorEngine, 'a' pairs multiply on ScalarE (activation with per-channel
scale) and accumulate with bf16 2x-mode adds on the VectorEngine, and 't'
pairs accumulate diag(k_tap) @ window matmuls in PSUM on the TensorEngine
with the diagonals built by the otherwise-idle GpSimd engine.  Heads run in
bf16.  Emission is staged so sample b's xcorr overlaps conv of samples
b+1/b+2, with heads lagging two samples.
"""

import sys

if "/opt/trn_rl_repo" not in sys.path:
    sys.path.insert(0, "/opt/trn_rl_repo")

from contextlib import ExitStack

import ml_dtypes
import numpy as np

import concourse.bass as bass
import concourse.tile as tile
from concourse import bacc, mybir
from concourse.bass_utils import run_bass_kernel_spmd

EPS = 1e-5
NCORES = 8
B, C, HID, OUT = 64, 256, 256, 10
BPC = B // NCORES  # samples per core
P = 128
KC = C // P  # channel chunks (2)
F32 = mybir.dt.float32
BF16 = mybir.dt.bfloat16
AF = mybir.ActivationFunctionType
OP = mybir.AluOpType

# xcorr engine per (b, cc) pair, index p = b*2 + cc
# 'v' = VectorE chain, 'a' = ScalarE-mult + VectorE-add,
# 'g' = GpSimd-mult + VectorE-add, 't' = TensorE diag (diags prebuilt on
# GpSimd), 's' = split pair: 13 taps TensorE + 12 taps VectorE chain
XC_ENGINE = ["t", "a", "v", "a",
             "t", "a", "v", "a",
             "t", "a", "v", "a",
             "t", "s", "s", "s"]

LAST_RESULTS = None  # BassKernelResults of the most recent run (for profiling)

_prog_cache = {}


def _emit(nc, tc, ctx, d):
    """Emit the per-core program.  d maps dram tensor name -> handle."""
    wp = ctx.enter_context(tc.tile_pool(name="weights", bufs=1))
    srp = ctx.enter_context(tc.tile_pool(name="srelu", bufs=1))
    krp = ctx.enter_context(tc.tile_pool(name="krelu", bufs=1))
    kp = ctx.enter_context(tc.tile_pool(name="kern", bufs=1))
    sp = ctx.enter_context(tc.tile_pool(name="search", bufs=8))
    featp = ctx.enter_context(tc.tile_pool(name="feat", bufs=10))
    tmpp = ctx.enter_context(tc.tile_pool(name="ftmp", bufs=8))
    diagp = ctx.enter_context(tc.tile_pool(name="diag", bufs=25))
    xrp = ctx.enter_context(tc.tile_pool(name="xrelu", bufs=6))
    outp = ctx.enter_context(tc.tile_pool(name="outs", bufs=1))
    ps_conv = ctx.enter_context(tc.tile_pool(name="ps_conv", bufs=3, space="PSUM"))
    ps_x = ctx.enter_context(tc.tile_pool(name="ps_x", bufs=3, space="PSUM"))
    ps_hd = ctx.enter_context(tc.tile_pool(name="ps_hd", bufs=2, space="PSUM"))

    # ---- weights / constants into SBUF (kernel branch first: conv_kernel
    # and the GpSimd diag builds run before the first search conv) ----
    csw_sb, ckw_sb, h1w_sb, h2w_sb = [], [], [], []
    csb_sb, ckb_sb, h1b_sb = [], [], []
    k_sb = []
    for kc in range(KC):
        t = kp.tile([P, BPC, 9, 9], BF16, tag=f"kin{kc}")
        nc.sync.dma_start(t[:], d["k_in"].ap()[kc])
        k_sb.append(t)
    for kc in range(KC):
        t = wp.tile([P, 9 * 2 * P], BF16, tag=f"ckw{kc}")
        for tap in range(9):
            nc.sync.dma_start(t[:, tap * 2 * P:(tap + 1) * 2 * P],
                              d["ckw"].ap()[kc, :, tap])
        ckw_sb.append(t)
    s0_sb = []
    for kc in range(KC):
        t = sp.tile([P, 31, 32], BF16, tag="sin", name=f"sin{kc}_0")
        nc.sync.dma_start(t[:], d["s_in"].ap()[kc, :, 0])
        s0_sb.append(t)
    for kc in range(KC):
        t = wp.tile([P, 9 * 2 * P], BF16, tag=f"csw{kc}")
        for tap in range(9):
            nc.sync.dma_start(t[:, tap * 2 * P:(tap + 1) * 2 * P],
                              d["csw"].ap()[kc, :, tap])
        csw_sb.append(t)
    for kc in range(KC):
        t = wp.tile([P, 2 * P], BF16, tag=f"h1w{kc}")
        nc.sync.dma_start(t[:], d["h1w"].ap()[kc])
        h1w_sb.append(t)
        t = wp.tile([P, OUT], BF16, tag=f"h2w{kc}")
        nc.sync.dma_start(t[:], d["h2w"].ap()[kc])
        h2w_sb.append(t)
    for mc in range(KC):
        t = wp.tile([P, 1], F32, tag=f"csb{mc}")
        nc.sync.dma_start(t[:], d["cs_bias"].ap()[mc])
        csb_sb.append(t)
        t = wp.tile([P, 1], F32, tag=f"ckb{mc}")
        nc.sync.dma_start(t[:], d["ck_bias"].ap()[mc])
        ckb_sb.append(t)
        t = wp.tile([P, 1], F32, tag=f"h1b{mc}")
        nc.sync.dma_start(t[:], d["h1_bias"].ap()[mc])
        h1b_sb.append(t)
    h2b_sb = wp.tile([OUT, 1], F32, tag="h2b")
    nc.sync.dma_start(h2b_sb[:], d["h2_bias"].ap())
    ident_sb = wp.tile([P, P], BF16, tag="ident")
    nc.sync.dma_start(ident_sb[:], d["ident"].ap())

    # ---- persistent activations ----
    krelu_sb = [krp.tile([P, BPC * 25], F32, tag=f"krelu{mc}", name=f"krelu{mc}") for mc in range(KC)]
    srelu_sb = [srp.tile([P, BPC, 29, 30], BF16, tag=f"srelu{mc}", name=f"srelu{mc}") for mc in range(KC)]

    out_sb = outp.tile([OUT, BPC * 625], F32, tag="osb")

    def kscalar(cc, b, tap):
        return krelu_sb[cc][:, b * 25 + tap:b * 25 + tap + 1]

    def winw(cc, b, tap, r0=0, nr=25):
        # [nr, 25] window of srelu, shifted by tap (rows stride 30)
        dy, dx = tap // 5, tap % 5
        return srelu_sb[cc][:, b, dy + r0:dy + r0 + nr, dx:dx + 25]

    def ftw(ft, r0=0, nr=25, w=25):
        # [nr, w] row window of the 26-stride feat tile
        return ft[:].rearrange("p (y x) -> p y x", x=26)[:, r0:r0 + nr, 0:w]

    SPLITS = ((0, 15), (15, 14))

    def conv_search(b, s_sb):
        for mc in range(KC):
            pss = [ps_conv.tile([P, nr, 29], F32, tag="pss", name=f"pss{b}_{mc}_{si}")
                   for si, (y0, nr) in enumerate(SPLITS)]
            i = 0
            for tap in range(9):
                dy, dx = tap // 3, tap % 3
                for kc in range(KC):
                    lhsT = csw_sb[kc][:, tap * 2 * P + mc * P:tap * 2 * P + (mc + 1) * P]
                    for si, (y0, nr) in enumerate(SPLITS):
                        rhs = s_sb[kc][:, y0 + dy:y0 + dy + nr, dx:dx + 29]
                        nc.tensor.matmul(pss[si][:], lhsT, rhs,
                                         start=(i == 0), stop=(i == 17))
                    i += 1
            for si, (y0, nr) in enumerate(SPLITS):
                nc.scalar.activation(srelu_sb[mc][:, b, y0:y0 + nr, 0:29], pss[si][:],
                                     AF.Relu, bias=csb_sb[mc][:])

    def conv_kernel():
        for mc in range(KC):
            psk = ps_conv.tile([P, BPC, 6, 6], F32, tag="pss", name=f"psk{mc}")
            i = 0
            for tap in range(9):
                dy, dx = tap // 3, tap % 3
                for kc in range(KC):
                    lhsT = ckw_sb[kc][:, tap * 2 * P + mc * P:tap * 2 * P + (mc + 1) * P]
                    rhs = k_sb[kc][:, :, dy:dy + 6, dx:dx + 6]
                    nc.tensor.matmul(psk[:], lhsT, rhs, start=(i == 0), stop=(i == 17))
                    i += 1
            nc.scalar.activation(krelu_sb[mc][:], psk[:, :, 0:5, 0:5], AF.Relu,
                                 bias=ckb_sb[mc][:])

    # prebuilt diag(k_tap) tiles for 't'/'s' pairs, built early on GpSimd
    diag_store = {}

    def build_diags():
        for b in range(BPC):
            for cc in range(KC):
                eng = XC_ENGINE[b * 2 + cc]
                if eng in ("t", "s"):
                    ntap = 25 if eng == "t" else 13
                    lst = []
                    for tap in range(ntap):
                        dg = diagp.tile([P, P], BF16, tag=f"dg{b}_{cc}",
                                        name=f"dg{b}_{cc}_{tap}")
                        nc.vector.tensor_scalar(dg[:], ident_sb[:],
                                                kscalar(cc, b, tap), None, OP.mult)
                        lst.append(dg)
                    diag_store[(b, cc)] = lst

    def make_pair(b, cc, eng):
        """Return (ft, [thunks]) -- thunks emit per-tap ops when called."""
        ops = []
        if eng in ("v", "a", "g"):
            ftA = featp.tile([P, 650], BF16, tag="ftA", name=f"ftA{b}_{cc}")
            ftB = featp.tile([P, 650], BF16, tag="ftB", name=f"ftB{b}_{cc}")

            def mul(dst, tap):
                if eng == "v":
                    nc.vector.tensor_scalar(ftw(dst), winw(cc, b, tap),
                                            kscalar(cc, b, tap), None, OP.mult)
                elif eng == "a":
                    nc.scalar.activation(ftw(dst), winw(cc, b, tap), AF.Copy,
                                         scale=kscalar(cc, b, tap))
                else:
                    nc.gpsimd.tensor_tensor(
                        ftw(dst), winw(cc, b, tap),
                        kscalar(cc, b, tap).broadcast_to([P, 25, 25]), OP.mult)

            ops.append(lambda: mul(ftA, 0))
            ops.append(lambda: mul(ftB, 1))
            for tap in range(2, 25):
                tgt = ftA if tap % 2 == 0 else ftB
                if eng == "v":
                    def step(tap=tap, tgt=tgt):
                        nc.vector.scalar_tensor_tensor(
                            ftw(tgt), winw(cc, b, tap), kscalar(cc, b, tap),
                            ftw(tgt), OP.mult, OP.add)
                else:
                    def step(tap=tap, tgt=tgt):
                        tmp = tmpp.tile([P, 650], BF16, tag="ftmp")
                        mul(tmp, tap)
                        nc.vector.tensor_tensor(ftw(tgt), ftw(tgt), ftw(tmp),
                                                OP.add)
                ops.append(step)
            ops.append(lambda: nc.vector.tensor_tensor(ftw(ftA), ftw(ftA),
                                                       ftw(ftB), OP.add))
            return ftA, ops

        if eng == "t":  # TensorE: diag(k_tap) @ windows accumulated in PSUM
            # two sequential row-halves, one PSUM bank at a time, so two
            # t-pairs can be in flight within the 2-buf psx ring
            ft = featp.tile([P, 650], BF16, tag="ftt", name=f"ftt{b}_{cc}")
            dgs = diag_store[(b, cc)]
            for hi, (r0, nr) in enumerate(((0, 13), (13, 12))):
                ps = ps_x.tile([P, nr, 25], F32, tag="psx", name=f"psx{b}_{cc}_{hi}")
                for tap in range(25):
                    def step(tap=tap, ps=ps, r0=r0, nr=nr):
                        nc.tensor.matmul(ps[:], dgs[tap][:], winw(cc, b, tap, r0, nr),
                                         start=(tap == 0), stop=(tap == 24))
                    ops.append(step)
                ops.append(lambda ps=ps, r0=r0, nr=nr: nc.scalar.activation(
                    ftw(ft, r0, nr), ps[:], AF.Copy))
            return ft, ops

        # 's': taps 0-12 on TensorE (sequential halves), taps 13-24 DVE 2-chain
        ftA = featp.tile([P, 650], BF16, tag="ftt", name=f"ftA{b}_{cc}")
        ftB = featp.tile([P, 650], BF16, tag="ftA", name=f"ftB{b}_{cc}")
        ftC = featp.tile([P, 650], BF16, tag="ftB", name=f"ftC{b}_{cc}")
        dgs = diag_store[(b, cc)]
        pe_ops, dve_ops = [], []
        for hi, (r0, nr) in enumerate(((0, 13), (13, 12))):
            ps = ps_x.tile([P, nr, 25], F32, tag="psx", name=f"psx{b}_{cc}_{hi}")
            for tap in range(13):
                def step(tap=tap, ps=ps, r0=r0, nr=nr):
                    nc.tensor.matmul(ps[:], dgs[tap][:], winw(cc, b, tap, r0, nr),
                                     start=(tap == 0), stop=(tap == 12))
                pe_ops.append(step)
            pe_ops.append(lambda ps=ps, r0=r0, nr=nr: nc.scalar.activation(
                ftw(ftA, r0, nr), ps[:], AF.Copy))
        dve_ops.append(lambda: nc.vector.tensor_scalar(
            ftw(ftB), winw(cc, b, 13), kscalar(cc, b, 13), None, OP.mult))
        dve_ops.append(lambda: nc.vector.tensor_scalar(
            ftw(ftC), winw(cc, b, 14), kscalar(cc, b, 14), None, OP.mult))
        for tap in range(15, 25):
            tgt = ftB if tap % 2 == 1 else ftC
            def step(tap=tap, tgt=tgt):
                nc.vector.scalar_tensor_tensor(
                    ftw(tgt), winw(cc, b, tap), kscalar(cc, b, tap),
                    ftw(tgt), OP.mult, OP.add)
            dve_ops.append(step)
        dve_ops.append(lambda: nc.vector.tensor_tensor(ftw(ftB), ftw(ftB),
                                                       ftw(ftC), OP.add))
        # interleave PE taps with DVE chain ops
        for i in range(max(len(pe_ops), len(dve_ops))):
            if i < len(pe_ops):
                ops.append(pe_ops[i])
            if i < len(dve_ops):
                ops.append(dve_ops[i])
        ops.append(lambda: nc.vector.tensor_tensor(ftw(ftA), ftw(ftA),
                                                   ftw(ftB), OP.add))
        return ftA, ops

    def xcorr(b):
        feat, plans = [], []
        for cc in range(KC):
            ft, ops = make_pair(b, cc, XC_ENGINE[b * 2 + cc])
            feat.append(ft)
            plans.append(ops)
        # round-robin emission so each engine's stream interleaves both pairs
        for i in range(max(len(p) for p in plans)):
            for ops in plans:
                if i < len(ops):
                    ops[i]()
        return feat

    def heads(b, feat):
        # head1: 1x1 conv + BN + ReLU (row-aligned splits over the 25x25 window)
        xr = []
        for mc in range(KC):
            t = xrp.tile([P, 626], BF16, tag="xr", name=f"xr{b}_{mc}")
            nc.vector.memset(t[:, 625:626], 0.0)
            xr.append(t)
        HS = ((0, 13), (13, 12))
        for mc in range(KC):
            ph = [ps_hd.tile([P, nr, 25], F32, tag="pshd", name=f"ph{b}_{mc}_{si}")
                  for si, (r0, nr) in enumerate(HS)]
            for kc in range(KC):
                lhsT = h1w_sb[kc][:, mc * P:(mc + 1) * P]
                for si, (r0, nr) in enumerate(HS):
                    nc.tensor.matmul(ph[si][:], lhsT, ftw(feat[kc], r0, nr),
                                     start=(kc == 0), stop=(kc == 1))
            for si, (r0, nr) in enumerate(HS):
                nc.scalar.activation(xr[mc][:, r0 * 25:(r0 + nr) * 25],
                                     ph[si][:], AF.Relu, bias=h1b_sb[mc][:])

        # head2: 1x1 conv + bias
        for o0, n, nv in ((0, 320, 320), (320, 306, 305)):
            po = ps_hd.tile([OUT, n], F32, tag="pshd", name=f"po{b}_{o0}")
            for kc in range(KC):
                nc.tensor.matmul(po[:], h2w_sb[kc][:],
                                 xr[kc][:, o0:o0 + n],
                                 start=(kc == 0), stop=(kc == 1))
            nc.scalar.activation(out_sb[:, b * 625 + o0:b * 625 + o0 + nv], po[:, 0:nv],
                                 AF.Identity, bias=h2b_sb[:])
        nc.sync.dma_start(d["out"].ap()[:, b * 625:(b + 1) * 625],
                          out_sb[:, b * 625:(b + 1) * 625])

    # ---- main pipeline: conv_kernel+diags first; xcorr lags conv by 1
    # (keeps relu ACTs ahead of the a-mult backlog in the ScalarE stream);
    # heads lag 3 ----
    conv_kernel()
    build_diags()
    feat_all = {}
    for b in range(BPC):
        if b == 0:
            s_sb = s0_sb
        else:
            s_sb = []
            for kc in range(KC):
                t = sp.tile([P, 31, 32], BF16, tag="sin", name=f"sin{kc}_{b}")
                nc.sync.dma_start(t[:], d["s_in"].ap()[kc, :, b])
                s_sb.append(t)
        conv_search(b, s_sb)
        if b >= 1:
            feat_all[b - 1] = xcorr(b - 1)
        if b >= 3:
            heads(b - 3, feat_all.pop(b - 3))
    feat_all[BPC - 1] = xcorr(BPC - 1)
    for b in range(BPC - 3, BPC):
        heads(b, feat_all.pop(b))


def _build_program():
    if "nc" in _prog_cache:
        return _prog_cache["nc"]
    nc = bacc.Bacc("TRN2", target_bir_lowering=False, debug=False,
                   num_devices=NCORES)
    d = {}
    d["s_in"] = nc.dram_tensor("s_in", [KC, P, BPC, 31, 32], BF16, kind="ExternalInput")
    d["k_in"] = nc.dram_tensor("k_in", [KC, P, BPC, 9, 9], BF16, kind="ExternalInput")
    d["csw"] = nc.dram_tensor("csw", [KC, P, 9, 2, P], BF16, kind="ExternalInput")
    d["ckw"] = nc.dram_tensor("ckw", [KC, P, 9, 2, P], BF16, kind="ExternalInput")
    d["cs_bias"] = nc.dram_tensor("cs_bias", [KC, P, 1], F32, kind="ExternalInput")
    d["ck_bias"] = nc.dram_tensor("ck_bias", [KC, P, 1], F32, kind="ExternalInput")
    d["h1w"] = nc.dram_tensor("h1w", [KC, P, 2, P], BF16, kind="ExternalInput")
    d["h1_bias"] = nc.dram_tensor("h1_bias", [KC, P, 1], F32, kind="ExternalInput")
    d["h2w"] = nc.dram_tensor("h2w", [KC, P, OUT], BF16, kind="ExternalInput")
    d["h2_bias"] = nc.dram_tensor("h2_bias", [OUT, 1], F32, kind="ExternalInput")
    d["ident"] = nc.dram_tensor("ident", [P, P], BF16, kind="ExternalInput")
    d["out"] = nc.dram_tensor("out", [OUT, BPC * 625], F32, kind="ExternalOutput")

    with tile.TileContext(nc) as tc:
        with ExitStack() as ctx:
            _emit(nc, tc, ctx, d)
    nc.compile()
    _prog_cache["nc"] = nc
    return nc


def kernel(**inputs):
    global LAST_RESULTS
    f32 = lambda x: np.ascontiguousarray(np.asarray(x), dtype=np.float32)
    kern, search = f32(inputs["kernel"]), f32(inputs["search"])

    # fold BN into conv weights / biases
    cks = f32(inputs["ck_g"]) / np.sqrt(f32(inputs["ck_v"]) + EPS)
    ckw_f = f32(inputs["ck_w"]) * cks[:, None, None, None]
    ckb = f32(inputs["ck_b"]) - f32(inputs["ck_m"]) * cks
    css = f32(inputs["cs_g"]) / np.sqrt(f32(inputs["cs_v"]) + EPS)
    csw_f = f32(inputs["cs_w"]) * css[:, None, None, None]
    csb = f32(inputs["cs_b"]) - f32(inputs["cs_m"]) * css
    h1s = f32(inputs["h_g"]) / np.sqrt(f32(inputs["h_v"]) + EPS)
    h1w_f = f32(inputs["h1_w"]) * h1s[:, None]
    h1b = f32(inputs["h_b"]) - f32(inputs["h_m"]) * h1s

    shared = {
        "csw": np.ascontiguousarray(
            csw_f.transpose(1, 2, 3, 0).reshape(KC, P, 9, 2, P)).astype(ml_dtypes.bfloat16),
        "ckw": np.ascontiguousarray(
            ckw_f.transpose(1, 2, 3, 0).reshape(KC, P, 9, 2, P)).astype(ml_dtypes.bfloat16),
        "cs_bias": csb.reshape(KC, P, 1),
        "ck_bias": ckb.reshape(KC, P, 1),
        "h1w": np.ascontiguousarray(
            h1w_f.transpose(1, 0).reshape(KC, P, 2, P)).astype(ml_dtypes.bfloat16),
        "h1_bias": h1b.reshape(KC, P, 1),
        "h2w": np.ascontiguousarray(
            f32(inputs["h2_w"]).transpose(1, 0).reshape(KC, P, OUT)).astype(ml_dtypes.bfloat16),
        "h2_bias": f32(inputs["h2_b"]).reshape(OUT, 1),
        "ident": np.eye(P, dtype=ml_dtypes.bfloat16),
    }
    in_maps = []
    for i in range(NCORES):
        sl = slice(i * BPC, (i + 1) * BPC)
        m = dict(shared)
        s_pad = np.zeros((KC, P, BPC, 31, 32), ml_dtypes.bfloat16)
        s_pad[..., :31] = search[sl].transpose(1, 0, 2, 3).reshape(KC, P, BPC, 31, 31)
        m["s_in"] = s_pad
        k_pad = np.zeros((KC, P, BPC, 9, 9), ml_dtypes.bfloat16)
        k_pad[..., :7, :7] = kern[sl].transpose(1, 0, 2, 3).reshape(KC, P, BPC, 7, 7)
        m["k_in"] = k_pad
        in_maps.append(m)

    nc = _build_program()
    res = run_bass_kernel_spmd(nc, in_maps, core_ids=list(range(NCORES)))
    LAST_RESULTS = res
    out = np.empty((B, OUT, 25, 25), dtype=np.float32)
    for i in range(NCORES):
        o = res.results[i]["out"].reshape(OUT, BPC, 25, 25)
        out[i * BPC:(i + 1) * BPC] = o.transpose(1, 0, 2, 3)
    return out


# revision 22
# speedup vs baseline: 1.0729x; 1.0729x over previous
"""Trainium2 Bass kernel for DepthwiseXCorr (SiamRPN-style) model.

Pipeline (per sample): conv3x3+BN+ReLU on kernel & search branches,
depthwise cross-correlation, 1x1 conv + BN + ReLU head, 1x1 conv + bias.

Sharding: data-parallel over batch across 8 NeuronCores (8 samples each),
weights replicated.  BN is folded into conv weights on the host.

Layout on device: channels on SBUF partitions (2 chunks of 128), spatial x
batch on the free dimension.  Convolutions run as 9 shifted matmuls in bf16
(split-row PSUM tiles interleaved so consecutive matmuls share weights).
The depthwise xcorr is elementwise work spread over three engines per
(sample, channel-chunk) pair: 'v' pairs run multiply-accumulate chains on
the VectorEngine, 'a' pairs multiply on ScalarE (activation with per-channel
scale) and accumulate with bf16 2x-mode adds on the VectorEngine, and 't'
pairs accumulate diag(k_tap) @ window matmuls in PSUM on the TensorEngine
with the diagonals built by the otherwise-idle GpSimd engine.  Heads run in
bf16.  Emission is staged so sample b's xcorr overlaps conv of samples
b+1/b+2, with heads lagging two samples.
"""

import sys

if "/opt/trn_rl_repo" not in sys.path:
    sys.path.insert(0, "/opt/trn_rl_repo")

from contextlib import ExitStack

import ml_dtypes
import numpy as np

import concourse.bass as bass
import concourse.tile as tile
from concourse import bacc, mybir
from concourse.bass_utils import run_bass_kernel_spmd

EPS = 1e-5
NCORES = 8
B, C, HID, OUT = 64, 256, 256, 10
BPC = B // NCORES  # samples per core
P = 128
KC = C // P  # channel chunks (2)
F32 = mybir.dt.float32
BF16 = mybir.dt.bfloat16
AF = mybir.ActivationFunctionType
OP = mybir.AluOpType

# xcorr engine per (b, cc) pair, index p = b*2 + cc
# 'v' = VectorE chain, 'a' = ScalarE-mult + VectorE-add,
# 'g' = GpSimd-mult + VectorE-add, 't' = TensorE diag (diags prebuilt on
# GpSimd), 's' = split pair: 13 taps TensorE + 12 taps VectorE chain
XC_ENGINE = ["t", "a", "v", "a",
             "t", "a", "v", "a",
             "t", "a", "v", "a",
             "t", "s", "s", "s"]

LAST_RESULTS = None  # BassKernelResults of the most recent run (for profiling)

_prog_cache = {}


def _emit(nc, tc, ctx, d):
    """Emit the per-core program.  d maps dram tensor name -> handle."""
    wp = ctx.enter_context(tc.tile_pool(name="weights", bufs=1))
    srp = ctx.enter_context(tc.tile_pool(name="srelu", bufs=1))
    krp = ctx.enter_context(tc.tile_pool(name="krelu", bufs=1))
    kp = ctx.enter_context(tc.tile_pool(name="kern", bufs=1))
    sp = ctx.enter_context(tc.tile_pool(name="search", bufs=8))
    featp = ctx.enter_context(tc.tile_pool(name="feat", bufs=10))
    tmpp = ctx.enter_context(tc.tile_pool(name="ftmp", bufs=8))
    diagp = ctx.enter_context(tc.tile_pool(name="diag", bufs=25))
    xrp = ctx.enter_context(tc.tile_pool(name="xrelu", bufs=6))
    outp = ctx.enter_context(tc.tile_pool(name="outs", bufs=1))
    ps_conv = ctx.enter_context(tc.tile_pool(name="ps_conv", bufs=3, space="PSUM"))
    ps_x = ctx.enter_context(tc.tile_pool(name="ps_x", bufs=3, space="PSUM"))
    ps_hd = ctx.enter_context(tc.tile_pool(name="ps_hd", bufs=2, space="PSUM"))

    # ---- weights / constants into SBUF (kernel branch first: conv_kernel
    # and the GpSimd diag builds run before the first search conv) ----
    csw_sb, ckw_sb, h1w_sb, h2w_sb = [], [], [], []
    csb_sb, ckb_sb, h1b_sb = [], [], []
    k_sb = []
    for kc in range(KC):
        t = kp.tile([P, BPC, 9, 9], BF16, tag=f"kin{kc}")
        nc.sync.dma_start(t[:], d["k_in"].ap()[kc])
        k_sb.append(t)
    for kc in range(KC):
        t = wp.tile([P, 9 * 2 * P], BF16, tag=f"ckw{kc}")
        nc.sync.dma_start(t[:], d["ckw"].ap()[kc])
        ckw_sb.append(t)
    ident_sb = wp.tile([P, P], BF16, tag="ident")
    nc.sync.dma_start(ident_sb[:], d["ident"].ap())
    for mc in range(KC):
        t = wp.tile([P, 1], F32, tag=f"ckb{mc}")
        nc.sync.dma_start(t[:], d["ck_bias"].ap()[mc])
        ckb_sb.append(t)
        t = wp.tile([P, 1], F32, tag=f"csb{mc}")
        nc.sync.dma_start(t[:], d["cs_bias"].ap()[mc])
        csb_sb.append(t)
        t = wp.tile([P, 1], F32, tag=f"h1b{mc}")
        nc.sync.dma_start(t[:], d["h1_bias"].ap()[mc])
        h1b_sb.append(t)
    h2b_sb = wp.tile([OUT, 1], F32, tag="h2b")
    nc.sync.dma_start(h2b_sb[:], d["h2_bias"].ap())
    s0_sb = []
    for kc in range(KC):
        t = sp.tile([P, 31, 32], BF16, tag="sin", name=f"sin{kc}_0")
        nc.sync.dma_start(t[:], d["s_in"].ap()[kc, :, 0])
        s0_sb.append(t)
    for kc in range(KC):
        t = wp.tile([P, 9 * 2 * P], BF16, tag=f"csw{kc}")
        nc.sync.dma_start(t[:], d["csw"].ap()[kc])
        csw_sb.append(t)
    for kc in range(KC):
        t = wp.tile([P, 2 * P], BF16, tag=f"h1w{kc}")
        nc.sync.dma_start(t[:], d["h1w"].ap()[kc])
        h1w_sb.append(t)
        t = wp.tile([P, OUT], BF16, tag=f"h2w{kc}")
        nc.sync.dma_start(t[:], d["h2w"].ap()[kc])
        h2w_sb.append(t)

    # ---- persistent activations ----
    krelu_sb = [krp.tile([P, BPC * 25], F32, tag=f"krelu{mc}", name=f"krelu{mc}") for mc in range(KC)]
    srelu_sb = [srp.tile([P, BPC, 29, 30], BF16, tag=f"srelu{mc}", name=f"srelu{mc}") for mc in range(KC)]

    out_sb = outp.tile([OUT, BPC * 625], F32, tag="osb")

    def kscalar(cc, b, tap):
        return krelu_sb[cc][:, b * 25 + tap:b * 25 + tap + 1]

    def winw(cc, b, tap, r0=0, nr=25):
        # [nr, 25] window of srelu, shifted by tap (rows stride 30)
        dy, dx = tap // 5, tap % 5
        return srelu_sb[cc][:, b, dy + r0:dy + r0 + nr, dx:dx + 25]

    def ftw(ft, r0=0, nr=25, w=25):
        # [nr, w] row window of the 26-stride feat tile
        return ft[:].rearrange("p (y x) -> p y x", x=26)[:, r0:r0 + nr, 0:w]

    SPLITS = ((0, 15), (15, 14))

    def conv_search(b, s_sb):
        for mc in range(KC):
            pss = [ps_conv.tile([P, nr, 29], F32, tag="pss", name=f"pss{b}_{mc}_{si}")
                   for si, (y0, nr) in enumerate(SPLITS)]
            i = 0
            for tap in range(9):
                dy, dx = tap // 3, tap % 3
                for kc in range(KC):
                    lhsT = csw_sb[kc][:, tap * 2 * P + mc * P:tap * 2 * P + (mc + 1) * P]
                    for si, (y0, nr) in enumerate(SPLITS):
                        rhs = s_sb[kc][:, y0 + dy:y0 + dy + nr, dx:dx + 29]
                        nc.tensor.matmul(pss[si][:], lhsT, rhs,
                                         start=(i == 0), stop=(i == 17))
                    i += 1
            for si, (y0, nr) in enumerate(SPLITS):
                nc.scalar.activation(srelu_sb[mc][:, b, y0:y0 + nr, 0:29], pss[si][:],
                                     AF.Relu, bias=csb_sb[mc][:])

    def conv_kernel():
        for mc in range(KC):
            psk = ps_conv.tile([P, BPC, 6, 6], F32, tag="pss", name=f"psk{mc}")
            i = 0
            for tap in range(9):
                dy, dx = tap // 3, tap % 3
                for kc in range(KC):
                    lhsT = ckw_sb[kc][:, tap * 2 * P + mc * P:tap * 2 * P + (mc + 1) * P]
                    rhs = k_sb[kc][:, :, dy:dy + 6, dx:dx + 6]
                    nc.tensor.matmul(psk[:], lhsT, rhs, start=(i == 0), stop=(i == 17))
                    i += 1
            nc.scalar.activation(krelu_sb[mc][:], psk[:, :, 0:5, 0:5], AF.Relu,
                                 bias=ckb_sb[mc][:])

    # prebuilt diag(k_tap) tiles for 't'/'s' pairs, built early on GpSimd
    diag_store = {}

    def build_diags():
        for b in range(BPC):
            for cc in range(KC):
                eng = XC_ENGINE[b * 2 + cc]
                if eng in ("t", "s"):
                    ntap = 25 if eng == "t" else 13
                    lst = []
                    for tap in range(ntap):
                        dg = diagp.tile([P, P], BF16, tag=f"dg{b}_{cc}",
                                        name=f"dg{b}_{cc}_{tap}")
                        nc.vector.tensor_scalar(dg[:], ident_sb[:],
                                                kscalar(cc, b, tap), None, OP.mult)
                        lst.append(dg)
                    diag_store[(b, cc)] = lst

    def make_pair(b, cc, eng):
        """Return (ft, [thunks]) -- thunks emit per-tap ops when called."""
        ops = []
        if eng in ("v", "a", "g"):
            ftA = featp.tile([P, 650], BF16, tag="ftA", name=f"ftA{b}_{cc}")
            ftB = featp.tile([P, 650], BF16, tag="ftB", name=f"ftB{b}_{cc}")

            def mul(dst, tap):
                if eng == "v":
                    nc.vector.tensor_scalar(ftw(dst), winw(cc, b, tap),
                                            kscalar(cc, b, tap), None, OP.mult)
                elif eng == "a":
                    nc.scalar.activation(ftw(dst), winw(cc, b, tap), AF.Copy,
                                         scale=kscalar(cc, b, tap))
                else:
                    nc.gpsimd.tensor_tensor(
                        ftw(dst), winw(cc, b, tap),
                        kscalar(cc, b, tap).broadcast_to([P, 25, 25]), OP.mult)

            ops.append(lambda: mul(ftA, 0))
            ops.append(lambda: mul(ftB, 1))
            for tap in range(2, 25):
                tgt = ftA if tap % 2 == 0 else ftB
                if eng == "v":
                    def step(tap=tap, tgt=tgt):
                        nc.vector.scalar_tensor_tensor(
                            ftw(tgt), winw(cc, b, tap), kscalar(cc, b, tap),
                            ftw(tgt), OP.mult, OP.add)
                else:
                    def step(tap=tap, tgt=tgt):
                        tmp = tmpp.tile([P, 650], BF16, tag="ftmp")
                        mul(tmp, tap)
                        nc.vector.tensor_tensor(ftw(tgt), ftw(tgt), ftw(tmp),
                                                OP.add)
                ops.append(step)
            ops.append(lambda: nc.vector.tensor_tensor(ftw(ftA), ftw(ftA),
                                                       ftw(ftB), OP.add))
            return ftA, ops

        if eng == "t":  # TensorE: diag(k_tap) @ windows accumulated in PSUM
            # two sequential row-halves, one PSUM bank at a time, so two
            # t-pairs can be in flight within the 2-buf psx ring
            ft = featp.tile([P, 650], BF16, tag="ftt", name=f"ftt{b}_{cc}")
            dgs = diag_store[(b, cc)]
            for hi, (r0, nr) in enumerate(((0, 13), (13, 12))):
                ps = ps_x.tile([P, nr, 25], F32, tag="psx", name=f"psx{b}_{cc}_{hi}")
                for tap in range(25):
                    def step(tap=tap, ps=ps, r0=r0, nr=nr):
                        nc.tensor.matmul(ps[:], dgs[tap][:], winw(cc, b, tap, r0, nr),
                                         start=(tap == 0), stop=(tap == 24))
                    ops.append(step)
                ops.append(lambda ps=ps, r0=r0, nr=nr: nc.scalar.activation(
                    ftw(ft, r0, nr), ps[:], AF.Copy))
            return ft, ops

        # 's': taps 0-12 on TensorE (sequential halves), taps 13-24 DVE 2-chain
        ftA = featp.tile([P, 650], BF16, tag="ftt", name=f"ftA{b}_{cc}")
        ftB = featp.tile([P, 650], BF16, tag="ftA", name=f"ftB{b}_{cc}")
        ftC = featp.tile([P, 650], BF16, tag="ftB", name=f"ftC{b}_{cc}")
        dgs = diag_store[(b, cc)]
        pe_ops, dve_ops = [], []
        for hi, (r0, nr) in enumerate(((0, 13), (13, 12))):
            ps = ps_x.tile([P, nr, 25], F32, tag="psx", name=f"psx{b}_{cc}_{hi}")
            for tap in range(13):
                def step(tap=tap, ps=ps, r0=r0, nr=nr):
                    nc.tensor.matmul(ps[:], dgs[tap][:], winw(cc, b, tap, r0, nr),
                                     start=(tap == 0), stop=(tap == 12))
                pe_ops.append(step)
            pe_ops.append(lambda ps=ps, r0=r0, nr=nr: nc.scalar.activation(
                ftw(ftA, r0, nr), ps[:], AF.Copy))
        dve_ops.append(lambda: nc.vector.tensor_scalar(
            ftw(ftB), winw(cc, b, 13), kscalar(cc, b, 13), None, OP.mult))
        dve_ops.append(lambda: nc.vector.tensor_scalar(
            ftw(ftC), winw(cc, b, 14), kscalar(cc, b, 14), None, OP.mult))
        for tap in range(15, 25):
            tgt = ftB if tap % 2 == 1 else ftC
            def step(tap=tap, tgt=tgt):
                nc.vector.scalar_tensor_tensor(
                    ftw(tgt), winw(cc, b, tap), kscalar(cc, b, tap),
                    ftw(tgt), OP.mult, OP.add)
            dve_ops.append(step)
        dve_ops.append(lambda: nc.vector.tensor_tensor(ftw(ftB), ftw(ftB),
                                                       ftw(ftC), OP.add))
        # interleave PE taps with DVE chain ops
        for i in range(max(len(pe_ops), len(dve_ops))):
            if i < len(pe_ops):
                ops.append(pe_ops[i])
            if i < len(dve_ops):
                ops.append(dve_ops[i])
        ops.append(lambda: nc.vector.tensor_tensor(ftw(ftA), ftw(ftA),
                                                   ftw(ftB), OP.add))
        return ftA, ops

    def xcorr(b):
        feat, plans = [], []
        for cc in range(KC):
            ft, ops = make_pair(b, cc, XC_ENGINE[b * 2 + cc])
            feat.append(ft)
            plans.append(ops)
        # round-robin emission so each engine's stream interleaves both pairs
        for i in range(max(len(p) for p in plans)):
            for ops in plans:
                if i < len(ops):
                    ops[i]()
        return feat

    def heads(b, feat):
        # head1: 1x1 conv + BN + ReLU (row-aligned splits over the 25x25 window)
        xr = []
        for mc in range(KC):
            t = xrp.tile([P, 626], BF16, tag="xr", name=f"xr{b}_{mc}")
            nc.vector.memset(t[:, 625:626], 0.0)
            xr.append(t)
        HS = ((0, 13), (13, 12))
        for mc in range(KC):
            ph = [ps_hd.tile([P, nr, 25], F32, tag="pshd", name=f"ph{b}_{mc}_{si}")
                  for si, (r0, nr) in enumerate(HS)]
            for kc in range(KC):
                lhsT = h1w_sb[kc][:, mc * P:(mc + 1) * P]
                for si, (r0, nr) in enumerate(HS):
                    nc.tensor.matmul(ph[si][:], lhsT, ftw(feat[kc], r0, nr),
                                     start=(kc == 0), stop=(kc == 1))
            for si, (r0, nr) in enumerate(HS):
                nc.scalar.activation(xr[mc][:, r0 * 25:(r0 + nr) * 25],
                                     ph[si][:], AF.Relu, bias=h1b_sb[mc][:])

        # head2: 1x1 conv + bias
        for o0, n, nv in ((0, 320, 320), (320, 306, 305)):
            po = ps_hd.tile([OUT, n], F32, tag="pshd", name=f"po{b}_{o0}")
            for kc in range(KC):
                nc.tensor.matmul(po[:], h2w_sb[kc][:],
                                 xr[kc][:, o0:o0 + n],
                                 start=(kc == 0), stop=(kc == 1))
            nc.scalar.activation(out_sb[:, b * 625 + o0:b * 625 + o0 + nv], po[:, 0:nv],
                                 AF.Identity, bias=h2b_sb[:])
        nc.sync.dma_start(d["out"].ap()[:, b * 625:(b + 1) * 625],
                          out_sb[:, b * 625:(b + 1) * 625])

    # ---- main pipeline: conv_kernel+diags first; xcorr lags conv by 1
    # (keeps relu ACTs ahead of the a-mult backlog in the ScalarE stream);
    # heads lag 3 ----
    conv_kernel()
    build_diags()
    feat_all = {}
    for b in range(BPC):
        if b == 0:
            s_sb = s0_sb
        else:
            s_sb = []
            for kc in range(KC):
                t = sp.tile([P, 31, 32], BF16, tag="sin", name=f"sin{kc}_{b}")
                nc.sync.dma_start(t[:], d["s_in"].ap()[kc, :, b])
                s_sb.append(t)
        conv_search(b, s_sb)
        if b >= 1:
            feat_all[b - 1] = xcorr(b - 1)
        if b >= 3:
            heads(b - 3, feat_all.pop(b - 3))
    feat_all[BPC - 1] = xcorr(BPC - 1)
    for b in range(BPC - 3, BPC):
        heads(b, feat_all.pop(b))


def _build_program():
    if "nc" in _prog_cache:
        return _prog_cache["nc"]
    nc = bacc.Bacc("TRN2", target_bir_lowering=False, debug=False,
                   num_devices=NCORES)
    d = {}
    d["s_in"] = nc.dram_tensor("s_in", [KC, P, BPC, 31, 32], BF16, kind="ExternalInput")
    d["k_in"] = nc.dram_tensor("k_in", [KC, P, BPC, 9, 9], BF16, kind="ExternalInput")
    d["csw"] = nc.dram_tensor("csw", [KC, P, 9, 2, P], BF16, kind="ExternalInput")
    d["ckw"] = nc.dram_tensor("ckw", [KC, P, 9, 2, P], BF16, kind="ExternalInput")
    d["cs_bias"] = nc.dram_tensor("cs_bias", [KC, P, 1], F32, kind="ExternalInput")
    d["ck_bias"] = nc.dram_tensor("ck_bias", [KC, P, 1], F32, kind="ExternalInput")
    d["h1w"] = nc.dram_tensor("h1w", [KC, P, 2, P], BF16, kind="ExternalInput")
    d["h1_bias"] = nc.dram_tensor("h1_bias", [KC, P, 1], F32, kind="ExternalInput")
    d["h2w"] = nc.dram_tensor("h2w", [KC, P, OUT], BF16, kind="ExternalInput")
    d["h2_bias"] = nc.dram_tensor("h2_bias", [OUT, 1], F32, kind="ExternalInput")
    d["ident"] = nc.dram_tensor("ident", [P, P], BF16, kind="ExternalInput")
    d["out"] = nc.dram_tensor("out", [OUT, BPC * 625], F32, kind="ExternalOutput")

    with tile.TileContext(nc) as tc:
        with ExitStack() as ctx:
            _emit(nc, tc, ctx, d)
    nc.compile()
    _prog_cache["nc"] = nc
    return nc


def kernel(**inputs):
    global LAST_RESULTS
    f32 = lambda x: np.ascontiguousarray(np.asarray(x), dtype=np.float32)
    kern, search = f32(inputs["kernel"]), f32(inputs["search"])

    # fold BN into conv weights / biases
    cks = f32(inputs["ck_g"]) / np.sqrt(f32(inputs["ck_v"]) + EPS)
    ckw_f = f32(inputs["ck_w"]) * cks[:, None, None, None]
    ckb = f32(inputs["ck_b"]) - f32(inputs["ck_m"]) * cks
    css = f32(inputs["cs_g"]) / np.sqrt(f32(inputs["cs_v"]) + EPS)
    csw_f = f32(inputs["cs_w"]) * css[:, None, None, None]
    csb = f32(inputs["cs_b"]) - f32(inputs["cs_m"]) * css
    h1s = f32(inputs["h_g"]) / np.sqrt(f32(inputs["h_v"]) + EPS)
    h1w_f = f32(inputs["h1_w"]) * h1s[:, None]
    h1b = f32(inputs["h_b"]) - f32(inputs["h_m"]) * h1s

    shared = {
        "csw": np.ascontiguousarray(
            csw_f.transpose(1, 2, 3, 0).reshape(KC, P, 9, 2, P)).astype(ml_dtypes.bfloat16),
        "ckw": np.ascontiguousarray(
            ckw_f.transpose(1, 2, 3, 0).reshape(KC, P, 9, 2, P)).astype(ml_dtypes.bfloat16),
        "cs_bias": csb.reshape(KC, P, 1),
        "ck_bias": ckb.reshape(KC, P, 1),
        "h1w": np.ascontiguousarray(
            h1w_f.transpose(1, 0).reshape(KC, P, 2, P)).astype(ml_dtypes.bfloat16),
        "h1_bias": h1b.reshape(KC, P, 1),
        "h2w": np.ascontiguousarray(
            f32(inputs["h2_w"]).transpose(1, 0).reshape(KC, P, OUT)).astype(ml_dtypes.bfloat16),
        "h2_bias": f32(inputs["h2_b"]).reshape(OUT, 1),
        "ident": np.eye(P, dtype=ml_dtypes.bfloat16),
    }
    in_maps = []
    for i in range(NCORES):
        sl = slice(i * BPC, (i + 1) * BPC)
        m = dict(shared)
        s_pad = np.zeros((KC, P, BPC, 31, 32), ml_dtypes.bfloat16)
        s_pad[..., :31] = search[sl].transpose(1, 0, 2, 3).reshape(KC, P, BPC, 31, 31)
        m["s_in"] = s_pad
        k_pad = np.zeros((KC, P, BPC, 9, 9), ml_dtypes.bfloat16)
        k_pad[..., :7, :7] = kern[sl].transpose(1, 0, 2, 3).reshape(KC, P, BPC, 7, 7)
        m["k_in"] = k_pad
        in_maps.append(m)

    nc = _build_program()
    res = run_bass_kernel_spmd(nc, in_maps, core_ids=list(range(NCORES)))
    LAST_RESULTS = res
    out = np.empty((B, OUT, 25, 25), dtype=np.float32)
    for i in range(NCORES):
        o = res.results[i]["out"].reshape(OUT, BPC, 25, 25)
        out[i * BPC:(i + 1) * BPC] = o.transpose(1, 0, 2, 3)
    return out


# revision 23
# speedup vs baseline: 1.0946x; 1.0202x over previous
"""Trainium2 Bass kernel for DepthwiseXCorr (SiamRPN-style) model.

Pipeline (per sample): conv3x3+BN+ReLU on kernel & search branches,
depthwise cross-correlation, 1x1 conv + BN + ReLU head, 1x1 conv + bias.

Sharding: data-parallel over batch across 8 NeuronCores (8 samples each),
weights replicated.  BN is folded into conv weights on the host.

Layout on device: channels on SBUF partitions (2 chunks of 128), spatial x
batch on the free dimension.  Convolutions run as 9 shifted matmuls in bf16
(split-row PSUM tiles interleaved so consecutive matmuls share weights).
The depthwise xcorr is elementwise work spread over three engines per
(sample, channel-chunk) pair: 'v' pairs run multiply-accumulate chains on
the VectorEngine, 'a' pairs multiply on ScalarE (activation with per-channel
scale) and accumulate with bf16 2x-mode adds on the VectorEngine, and 't'
pairs accumulate diag(k_tap) @ window matmuls in PSUM on the TensorEngine
with the diagonals built by the otherwise-idle GpSimd engine.  Heads run in
bf16.  Emission is staged so sample b's xcorr overlaps conv of samples
b+1/b+2, with heads lagging two samples.
"""

import sys

if "/opt/trn_rl_repo" not in sys.path:
    sys.path.insert(0, "/opt/trn_rl_repo")

from contextlib import ExitStack

import ml_dtypes
import numpy as np

import concourse.bass as bass
import concourse.tile as tile
from concourse import bacc, mybir
from concourse.bass_utils import run_bass_kernel_spmd

EPS = 1e-5
NCORES = 8
B, C, HID, OUT = 64, 256, 256, 10
BPC = B // NCORES  # samples per core
P = 128
KC = C // P  # channel chunks (2)
F32 = mybir.dt.float32
BF16 = mybir.dt.bfloat16
AF = mybir.ActivationFunctionType
OP = mybir.AluOpType

# xcorr engine per (b, cc) pair, index p = b*2 + cc
# 'v' = VectorE chain, 'a' = ScalarE-mult + VectorE-add,
# 'g' = GpSimd-mult + VectorE-add, 't' = TensorE diag (diags prebuilt on
# GpSimd), 's' = split pair: 13 taps TensorE + 12 taps VectorE chain
XC_ENGINE = ["t", "a", "v", "a",
             "t", "a", "v", "a",
             "t", "a", "v", "a",
             "t", "s", "s", "s"]

LAST_RESULTS = None  # BassKernelResults of the most recent run (for profiling)

_prog_cache = {}


def _emit(nc, tc, ctx, d):
    """Emit the per-core program.  d maps dram tensor name -> handle."""
    wp = ctx.enter_context(tc.tile_pool(name="weights", bufs=1))
    srp = ctx.enter_context(tc.tile_pool(name="srelu", bufs=1))
    krp = ctx.enter_context(tc.tile_pool(name="krelu", bufs=1))
    kp = ctx.enter_context(tc.tile_pool(name="kern", bufs=1))
    sp = ctx.enter_context(tc.tile_pool(name="search", bufs=8))
    featp = ctx.enter_context(tc.tile_pool(name="feat", bufs=12))
    tmpp = ctx.enter_context(tc.tile_pool(name="ftmp", bufs=8))
    diagp = ctx.enter_context(tc.tile_pool(name="diag", bufs=25))
    xrp = ctx.enter_context(tc.tile_pool(name="xrelu", bufs=8))
    outp = ctx.enter_context(tc.tile_pool(name="outs", bufs=1))
    ps_conv = ctx.enter_context(tc.tile_pool(name="ps_conv", bufs=3, space="PSUM"))
    ps_x = ctx.enter_context(tc.tile_pool(name="ps_x", bufs=3, space="PSUM"))
    ps_hd = ctx.enter_context(tc.tile_pool(name="ps_hd", bufs=2, space="PSUM"))

    # ---- weights / constants into SBUF (kernel branch first: conv_kernel
    # and the GpSimd diag builds run before the first search conv) ----
    csw_sb, ckw_sb, h1w_sb, h2w_sb = [], [], [], []
    csb_sb, ckb_sb, h1b_sb = [], [], []
    k_sb = []
    for kc in range(KC):
        t = kp.tile([P, BPC, 9, 9], BF16, tag=f"kin{kc}")
        nc.sync.dma_start(t[:], d["k_in"].ap()[kc])
        k_sb.append(t)
    for kc in range(KC):
        t = wp.tile([P, 9 * 2 * P], BF16, tag=f"ckw{kc}")
        nc.sync.dma_start(t[:], d["ckw"].ap()[kc])
        ckw_sb.append(t)
    ident_sb = wp.tile([P, P], BF16, tag="ident")
    nc.sync.dma_start(ident_sb[:], d["ident"].ap())
    for mc in range(KC):
        t = wp.tile([P, 1], F32, tag=f"ckb{mc}")
        nc.sync.dma_start(t[:], d["ck_bias"].ap()[mc])
        ckb_sb.append(t)
        t = wp.tile([P, 1], F32, tag=f"csb{mc}")
        nc.sync.dma_start(t[:], d["cs_bias"].ap()[mc])
        csb_sb.append(t)
        t = wp.tile([P, 1], F32, tag=f"h1b{mc}")
        nc.sync.dma_start(t[:], d["h1_bias"].ap()[mc])
        h1b_sb.append(t)
    h2b_sb = wp.tile([OUT, 1], F32, tag="h2b")
    nc.sync.dma_start(h2b_sb[:], d["h2_bias"].ap())
    s0_sb = []
    for kc in range(KC):
        t = sp.tile([P, 31, 32], BF16, tag="sin", name=f"sin{kc}_0")
        nc.sync.dma_start(t[:], d["s_in"].ap()[kc, :, 0])
        s0_sb.append(t)
    for kc in range(KC):
        t = wp.tile([P, 9 * 2 * P], BF16, tag=f"csw{kc}")
        nc.sync.dma_start(t[:], d["csw"].ap()[kc])
        csw_sb.append(t)
    for kc in range(KC):
        t = wp.tile([P, 2 * P], BF16, tag=f"h1w{kc}")
        nc.sync.dma_start(t[:], d["h1w"].ap()[kc])
        h1w_sb.append(t)
        t = wp.tile([P, OUT], BF16, tag=f"h2w{kc}")
        nc.sync.dma_start(t[:], d["h2w"].ap()[kc])
        h2w_sb.append(t)

    # ---- persistent activations ----
    krelu_sb = [krp.tile([P, BPC * 25], F32, tag=f"krelu{mc}", name=f"krelu{mc}") for mc in range(KC)]
    srelu_sb = [srp.tile([P, BPC, 29, 30], BF16, tag=f"srelu{mc}", name=f"srelu{mc}") for mc in range(KC)]

    out_sb = outp.tile([OUT, BPC * 625], F32, tag="osb")

    def kscalar(cc, b, tap):
        return krelu_sb[cc][:, b * 25 + tap:b * 25 + tap + 1]

    def winw(cc, b, tap, r0=0, nr=25):
        # [nr, 25] window of srelu, shifted by tap (rows stride 30)
        dy, dx = tap // 5, tap % 5
        return srelu_sb[cc][:, b, dy + r0:dy + r0 + nr, dx:dx + 25]

    def ftw(ft, r0=0, nr=25, w=25):
        # [nr, w] row window of the 26-stride feat tile
        return ft[:].rearrange("p (y x) -> p y x", x=26)[:, r0:r0 + nr, 0:w]

    SPLITS = ((0, 15), (15, 14))

    def conv_search(b, s_sb):
        for mc in range(KC):
            pss = [ps_conv.tile([P, nr, 29], F32, tag="pss", name=f"pss{b}_{mc}_{si}")
                   for si, (y0, nr) in enumerate(SPLITS)]
            i = 0
            for tap in range(9):
                dy, dx = tap // 3, tap % 3
                for kc in range(KC):
                    lhsT = csw_sb[kc][:, tap * 2 * P + mc * P:tap * 2 * P + (mc + 1) * P]
                    for si, (y0, nr) in enumerate(SPLITS):
                        rhs = s_sb[kc][:, y0 + dy:y0 + dy + nr, dx:dx + 29]
                        nc.tensor.matmul(pss[si][:], lhsT, rhs,
                                         start=(i == 0), stop=(i == 17))
                    i += 1
            for si, (y0, nr) in enumerate(SPLITS):
                nc.scalar.activation(srelu_sb[mc][:, b, y0:y0 + nr, 0:29], pss[si][:],
                                     AF.Relu, bias=csb_sb[mc][:])

    def conv_kernel():
        for mc in range(KC):
            psk = ps_conv.tile([P, BPC, 6, 6], F32, tag="pss", name=f"psk{mc}")
            i = 0
            for tap in range(9):
                dy, dx = tap // 3, tap % 3
                for kc in range(KC):
                    lhsT = ckw_sb[kc][:, tap * 2 * P + mc * P:tap * 2 * P + (mc + 1) * P]
                    rhs = k_sb[kc][:, :, dy:dy + 6, dx:dx + 6]
                    nc.tensor.matmul(psk[:], lhsT, rhs, start=(i == 0), stop=(i == 17))
                    i += 1
            nc.scalar.activation(krelu_sb[mc][:], psk[:, :, 0:5, 0:5], AF.Relu,
                                 bias=ckb_sb[mc][:])

    # prebuilt diag(k_tap) tiles for 't'/'s' pairs, built early on GpSimd
    diag_store = {}

    def build_diags():
        for b in range(BPC):
            for cc in range(KC):
                eng = XC_ENGINE[b * 2 + cc]
                if eng in ("t", "s"):
                    ntap = 25 if eng == "t" else 13
                    lst = []
                    for tap in range(ntap):
                        dg = diagp.tile([P, P], BF16, tag=f"dg{b}_{cc}",
                                        name=f"dg{b}_{cc}_{tap}")
                        nc.vector.tensor_scalar(dg[:], ident_sb[:],
                                                kscalar(cc, b, tap), None, OP.mult)
                        lst.append(dg)
                    diag_store[(b, cc)] = lst

    def make_pair(b, cc, eng):
        """Return (ft, [thunks]) -- thunks emit per-tap ops when called."""
        ops = []
        if eng in ("v", "a", "g"):
            ftA = featp.tile([P, 650], BF16, tag="ftA", name=f"ftA{b}_{cc}")
            ftB = featp.tile([P, 650], BF16, tag="ftB", name=f"ftB{b}_{cc}")

            def mul(dst, tap):
                if eng == "v":
                    nc.vector.tensor_scalar(ftw(dst), winw(cc, b, tap),
                                            kscalar(cc, b, tap), None, OP.mult)
                elif eng == "a":
                    nc.scalar.activation(ftw(dst), winw(cc, b, tap), AF.Copy,
                                         scale=kscalar(cc, b, tap))
                else:
                    nc.gpsimd.tensor_tensor(
                        ftw(dst), winw(cc, b, tap),
                        kscalar(cc, b, tap).broadcast_to([P, 25, 25]), OP.mult)

            ops.append(lambda: mul(ftA, 0))
            ops.append(lambda: mul(ftB, 1))
            for tap in range(2, 25):
                tgt = ftA if tap % 2 == 0 else ftB
                if eng == "v":
                    def step(tap=tap, tgt=tgt):
                        nc.vector.scalar_tensor_tensor(
                            ftw(tgt), winw(cc, b, tap), kscalar(cc, b, tap),
                            ftw(tgt), OP.mult, OP.add)
                else:
                    def step(tap=tap, tgt=tgt):
                        tmp = tmpp.tile([P, 650], BF16, tag="ftmp")
                        mul(tmp, tap)
                        nc.vector.tensor_tensor(ftw(tgt), ftw(tgt), ftw(tmp),
                                                OP.add)
                ops.append(step)
            ops.append(lambda: nc.vector.tensor_tensor(ftw(ftA), ftw(ftA),
                                                       ftw(ftB), OP.add))
            return ftA, ops

        if eng == "t":  # TensorE: diag(k_tap) @ windows accumulated in PSUM
            # two sequential row-halves, one PSUM bank at a time, so two
            # t-pairs can be in flight within the 2-buf psx ring
            ft = featp.tile([P, 650], BF16, tag="ftt", name=f"ftt{b}_{cc}")
            dgs = diag_store[(b, cc)]
            for hi, (r0, nr) in enumerate(((0, 13), (13, 12))):
                ps = ps_x.tile([P, nr, 25], F32, tag="psx", name=f"psx{b}_{cc}_{hi}")
                for tap in range(25):
                    def step(tap=tap, ps=ps, r0=r0, nr=nr):
                        nc.tensor.matmul(ps[:], dgs[tap][:], winw(cc, b, tap, r0, nr),
                                         start=(tap == 0), stop=(tap == 24))
                    ops.append(step)
                ops.append(lambda ps=ps, r0=r0, nr=nr: nc.scalar.activation(
                    ftw(ft, r0, nr), ps[:], AF.Copy))
            return ft, ops

        # 's': taps 0-12 on TensorE (sequential halves), taps 13-24 DVE 2-chain
        ftA = featp.tile([P, 650], BF16, tag="ftt", name=f"ftA{b}_{cc}")
        ftB = featp.tile([P, 650], BF16, tag="ftA", name=f"ftB{b}_{cc}")
        ftC = featp.tile([P, 650], BF16, tag="ftB", name=f"ftC{b}_{cc}")
        dgs = diag_store[(b, cc)]
        pe_ops, dve_ops = [], []
        for hi, (r0, nr) in enumerate(((0, 13), (13, 12))):
            ps = ps_x.tile([P, nr, 25], F32, tag="psx", name=f"psx{b}_{cc}_{hi}")
            for tap in range(13):
                def step(tap=tap, ps=ps, r0=r0, nr=nr):
                    nc.tensor.matmul(ps[:], dgs[tap][:], winw(cc, b, tap, r0, nr),
                                     start=(tap == 0), stop=(tap == 12))
                pe_ops.append(step)
            pe_ops.append(lambda ps=ps, r0=r0, nr=nr: nc.scalar.activation(
                ftw(ftA, r0, nr), ps[:], AF.Copy))
        dve_ops.append(lambda: nc.vector.tensor_scalar(
            ftw(ftB), winw(cc, b, 13), kscalar(cc, b, 13), None, OP.mult))
        dve_ops.append(lambda: nc.vector.tensor_scalar(
            ftw(ftC), winw(cc, b, 14), kscalar(cc, b, 14), None, OP.mult))
        for tap in range(15, 25):
            tgt = ftB if tap % 2 == 1 else ftC
            def step(tap=tap, tgt=tgt):
                nc.vector.scalar_tensor_tensor(
                    ftw(tgt), winw(cc, b, tap), kscalar(cc, b, tap),
                    ftw(tgt), OP.mult, OP.add)
            dve_ops.append(step)
        dve_ops.append(lambda: nc.vector.tensor_tensor(ftw(ftB), ftw(ftB),
                                                       ftw(ftC), OP.add))
        # interleave PE taps with DVE chain ops
        for i in range(max(len(pe_ops), len(dve_ops))):
            if i < len(pe_ops):
                ops.append(pe_ops[i])
            if i < len(dve_ops):
                ops.append(dve_ops[i])
        ops.append(lambda: nc.vector.tensor_tensor(ftw(ftA), ftw(ftA),
                                                   ftw(ftB), OP.add))
        return ftA, ops

    def xcorr(b):
        feat, plans = [], []
        for cc in range(KC):
            ft, ops = make_pair(b, cc, XC_ENGINE[b * 2 + cc])
            feat.append(ft)
            plans.append(ops)
        # round-robin emission so each engine's stream interleaves both pairs
        for i in range(max(len(p) for p in plans)):
            for ops in plans:
                if i < len(ops):
                    ops[i]()
        return feat

    def heads(b, feat):
        # head1: 1x1 conv + BN + ReLU (row-aligned splits over the 25x25 window)
        xr = []
        for mc in range(KC):
            t = xrp.tile([P, 626], BF16, tag="xr", name=f"xr{b}_{mc}")
            nc.vector.memset(t[:, 625:626], 0.0)
            xr.append(t)
        HS = ((0, 13), (13, 12))
        for mc in range(KC):
            ph = [ps_hd.tile([P, nr, 25], F32, tag="pshd", name=f"ph{b}_{mc}_{si}")
                  for si, (r0, nr) in enumerate(HS)]
            for kc in range(KC):
                lhsT = h1w_sb[kc][:, mc * P:(mc + 1) * P]
                for si, (r0, nr) in enumerate(HS):
                    nc.tensor.matmul(ph[si][:], lhsT, ftw(feat[kc], r0, nr),
                                     start=(kc == 0), stop=(kc == 1))
            for si, (r0, nr) in enumerate(HS):
                nc.scalar.activation(xr[mc][:, r0 * 25:(r0 + nr) * 25],
                                     ph[si][:], AF.Relu, bias=h1b_sb[mc][:])

        # head2: 1x1 conv + bias
        for o0, n, nv in ((0, 320, 320), (320, 306, 305)):
            po = ps_hd.tile([OUT, n], F32, tag="pshd", name=f"po{b}_{o0}")
            for kc in range(KC):
                nc.tensor.matmul(po[:], h2w_sb[kc][:],
                                 xr[kc][:, o0:o0 + n],
                                 start=(kc == 0), stop=(kc == 1))
            nc.scalar.activation(out_sb[:, b * 625 + o0:b * 625 + o0 + nv], po[:, 0:nv],
                                 AF.Identity, bias=h2b_sb[:])
        nc.sync.dma_start(d["out"].ap()[:, b * 625:(b + 1) * 625],
                          out_sb[:, b * 625:(b + 1) * 625])

    # ---- main pipeline: conv_kernel+diags first; xcorr lags conv by 1
    # (keeps relu ACTs ahead of the a-mult backlog in the ScalarE stream);
    # heads lag 3 ----
    conv_kernel()
    build_diags()
    feat_all = {}
    for b in range(BPC):
        if b == 0:
            s_sb = s0_sb
        else:
            s_sb = []
            for kc in range(KC):
                t = sp.tile([P, 31, 32], BF16, tag="sin", name=f"sin{kc}_{b}")
                nc.sync.dma_start(t[:], d["s_in"].ap()[kc, :, b])
                s_sb.append(t)
        conv_search(b, s_sb)
        if b >= 1:
            feat_all[b - 1] = xcorr(b - 1)
        if b >= 4:
            heads(b - 4, feat_all.pop(b - 4))
    feat_all[BPC - 1] = xcorr(BPC - 1)
    for b in range(BPC - 4, BPC):
        heads(b, feat_all.pop(b))


def _build_program():
    if "nc" in _prog_cache:
        return _prog_cache["nc"]
    nc = bacc.Bacc("TRN2", target_bir_lowering=False, debug=False,
                   num_devices=NCORES)
    d = {}
    d["s_in"] = nc.dram_tensor("s_in", [KC, P, BPC, 31, 32], BF16, kind="ExternalInput")
    d["k_in"] = nc.dram_tensor("k_in", [KC, P, BPC, 9, 9], BF16, kind="ExternalInput")
    d["csw"] = nc.dram_tensor("csw", [KC, P, 9, 2, P], BF16, kind="ExternalInput")
    d["ckw"] = nc.dram_tensor("ckw", [KC, P, 9, 2, P], BF16, kind="ExternalInput")
    d["cs_bias"] = nc.dram_tensor("cs_bias", [KC, P, 1], F32, kind="ExternalInput")
    d["ck_bias"] = nc.dram_tensor("ck_bias", [KC, P, 1], F32, kind="ExternalInput")
    d["h1w"] = nc.dram_tensor("h1w", [KC, P, 2, P], BF16, kind="ExternalInput")
    d["h1_bias"] = nc.dram_tensor("h1_bias", [KC, P, 1], F32, kind="ExternalInput")
    d["h2w"] = nc.dram_tensor("h2w", [KC, P, OUT], BF16, kind="ExternalInput")
    d["h2_bias"] = nc.dram_tensor("h2_bias", [OUT, 1], F32, kind="ExternalInput")
    d["ident"] = nc.dram_tensor("ident", [P, P], BF16, kind="ExternalInput")
    d["out"] = nc.dram_tensor("out", [OUT, BPC * 625], F32, kind="ExternalOutput")

    with tile.TileContext(nc) as tc:
        with ExitStack() as ctx:
            _emit(nc, tc, ctx, d)
    nc.compile()
    _prog_cache["nc"] = nc
    return nc


def kernel(**inputs):
    global LAST_RESULTS
    f32 = lambda x: np.ascontiguousarray(np.asarray(x), dtype=np.float32)
    kern, search = f32(inputs["kernel"]), f32(inputs["search"])

    # fold BN into conv weights / biases
    cks = f32(inputs["ck_g"]) / np.sqrt(f32(inputs["ck_v"]) + EPS)
    ckw_f = f32(inputs["ck_w"]) * cks[:, None, None, None]
    ckb = f32(inputs["ck_b"]) - f32(inputs["ck_m"]) * cks
    css = f32(inputs["cs_g"]) / np.sqrt(f32(inputs["cs_v"]) + EPS)
    csw_f = f32(inputs["cs_w"]) * css[:, None, None, None]
    csb = f32(inputs["cs_b"]) - f32(inputs["cs_m"]) * css
    h1s = f32(inputs["h_g"]) / np.sqrt(f32(inputs["h_v"]) + EPS)
    h1w_f = f32(inputs["h1_w"]) * h1s[:, None]
    h1b = f32(inputs["h_b"]) - f32(inputs["h_m"]) * h1s

    shared = {
        "csw": np.ascontiguousarray(
            csw_f.transpose(1, 2, 3, 0).reshape(KC, P, 9, 2, P)).astype(ml_dtypes.bfloat16),
        "ckw": np.ascontiguousarray(
            ckw_f.transpose(1, 2, 3, 0).reshape(KC, P, 9, 2, P)).astype(ml_dtypes.bfloat16),
        "cs_bias": csb.reshape(KC, P, 1),
        "ck_bias": ckb.reshape(KC, P, 1),
        "h1w": np.ascontiguousarray(
            h1w_f.transpose(1, 0).reshape(KC, P, 2, P)).astype(ml_dtypes.bfloat16),
        "h1_bias": h1b.reshape(KC, P, 1),
        "h2w": np.ascontiguousarray(
            f32(inputs["h2_w"]).transpose(1, 0).reshape(KC, P, OUT)).astype(ml_dtypes.bfloat16),
        "h2_bias": f32(inputs["h2_b"]).reshape(OUT, 1),
        "ident": np.eye(P, dtype=ml_dtypes.bfloat16),
    }
    in_maps = []
    for i in range(NCORES):
        sl = slice(i * BPC, (i + 1) * BPC)
        m = dict(shared)
        s_pad = np.zeros((KC, P, BPC, 31, 32), ml_dtypes.bfloat16)
        s_pad[..., :31] = search[sl].transpose(1, 0, 2, 3).reshape(KC, P, BPC, 31, 31)
        m["s_in"] = s_pad
        k_pad = np.zeros((KC, P, BPC, 9, 9), ml_dtypes.bfloat16)
        k_pad[..., :7, :7] = kern[sl].transpose(1, 0, 2, 3).reshape(KC, P, BPC, 7, 7)
        m["k_in"] = k_pad
        in_maps.append(m)

    nc = _build_program()
    res = run_bass_kernel_spmd(nc, in_maps, core_ids=list(range(NCORES)))
    LAST_RESULTS = res
    out = np.empty((B, OUT, 25, 25), dtype=np.float32)
    for i in range(NCORES):
        o = res.results[i]["out"].reshape(OUT, BPC, 25, 25)
        out[i * BPC:(i + 1) * BPC] = o.transpose(1, 0, 2, 3)
    return out
